# revision 9
# baseline (speedup 1.0000x reference)
"""ContactLoss KNN kernel for 8 TRN2 NeuronCores.

Strategy (data parallel over batch B=8, one batch per core):
  Device (per core): computes, for every hand vertex i (778, padded 896) and
  every 128-object chunk c (50000 objects padded to 51200 = 400 chunks), the
  chunk-max of the NEGATED shifted squared distance
        d'[i,j] = 2*h_i.o_j - ||o_j||^2   ( = rx_i - dist2[i,j] )
  via TensorE matmuls (bf16 hi/lo split, K=12 rows, fp32 PSUM accumulate)
  and VectorE tensor_reduce(max) -> table [896, 400] fp32.

  Host: for each hand vertex, the winning chunk (and any chunk within a
  tolerance band DELTA, covering the bf16-split noise) is re-evaluated
  exactly in fp32 (emulating the jax reference op order) to produce the
  exact argmin index and min distance. All remaining reference math
  (gather, losses) is trivial [8,778]-sized numpy work.
"""

import numpy as np
import ml_dtypes

B, NH, NO = 8, 778, 50000
NHP = 896            # 7 * 128 hand-vertex padding
NBLK = 25            # blocks of 2048 objects
BLK = 2048
NOP = NBLK * BLK     # 51200 padded objects
CHUNK = 64           # table granularity
NCHUNK = NOP // CHUNK  # 400
KROWS = 12           # 9 product rows + 3 ry rows
NTILE = NHP // 128   # 7 hand tiles

DELTA = np.float32(4e-3)    # fixed part of band: must exceed 2x worst-case PE noise
BF16_REL = np.float32(2.0 ** -8)  # adaptive part: 2x bf16 quantization of table values

# block scheduling: every DIRECT_EVERY-th block is reduced directly on DVE from
# PSUM (fp32, 1x); the rest are cast to bf16 by ScalarE and max-tree-reduced on
# DVE at 2x rate. Balances DVE vs ScalarE occupancy.
DIRECT_EVERY = 5

CONTACT_THRESH = 0.025
COLLISION_THRESH = 0.025

_BF16 = ml_dtypes.bfloat16

_cached = {}


def _build_nc():
    from contextlib import ExitStack
    import concourse.bacc as bacc
    import concourse.tile as tile
    from concourse import mybir

    nc = bacc.Bacc(
        "TRN2",
        target_bir_lowering=False,
        debug=False,
        enable_asserts=False,
        num_devices=8,
    )
    lhs = nc.dram_tensor("lhs", [KROWS, NHP], mybir.dt.bfloat16, kind="ExternalInput")
    rhs = nc.dram_tensor("rhs", [KROWS, NOP], mybir.dt.bfloat16, kind="ExternalInput")
    table = nc.dram_tensor(
        "table", [NHP, NCHUNK], mybir.dt.bfloat16, kind="ExternalOutput"
    )

    G = BLK // CHUNK  # 16 chunks per block

    with tile.TileContext(nc) as tc, ExitStack() as ctx:
        sing = ctx.enter_context(tc.tile_pool(name="sing", bufs=1))
        tabp = ctx.enter_context(tc.tile_pool(name="tabp", bufs=2))
        bfp = ctx.enter_context(tc.tile_pool(name="bfp", bufs=4))
        scr = ctx.enter_context(tc.tile_pool(name="scr", bufs=2))
        psum = ctx.enter_context(tc.tile_pool(name="psum", bufs=2, space="PSUM"))

        lhs_sb = sing.tile([KROWS, NHP], mybir.dt.bfloat16, tag="lhs")
        nc.sync.dma_start(out=lhs_sb, in_=lhs.ap())
        rhs_sb = sing.tile([KROWS, NOP], mybir.dt.bfloat16, tag="rhs")
        nc.sync.dma_start(out=rhs_sb, in_=rhs.ap())

        for t in range(NTILE):
            tab = tabp.tile([128, NCHUNK], mybir.dt.bfloat16, tag="tab")
            lt = lhs_sb[:, t * 128 : (t + 1) * 128]
            for b in range(NBLK):
                ps = psum.tile([128, BLK], mybir.dt.float32, tag="ps")
                for j in range(4):
                    nc.tensor.matmul(
                        ps[:, j * 512 : (j + 1) * 512],
                        lt,
                        rhs_sb[:, b * BLK + j * 512 : b * BLK + (j + 1) * 512],
                        start=True,
                        stop=True,
                    )
                tslice = tab[:, b * G : (b + 1) * G]
                if b % DIRECT_EVERY == 0:
                    # direct fp32 reduce from PSUM on VectorE (1x rate)
                    nc.vector.tensor_reduce(
                        out=tslice,
                        in_=ps.rearrange("p (g f) -> p g f", f=CHUNK),
                        axis=mybir.AxisListType.X,
                        op=mybir.AluOpType.max,
                    )
                else:
                    # ScalarE casts PSUM->SBUF bf16; VectorE bf16 max-tree at 2x
                    bf = bfp.tile([128, BLK], mybir.dt.bfloat16, tag="bf")
                    nc.scalar.copy(out=bf, in_=ps)
                    sa = scr.tile([128, G, CHUNK // 2], mybir.dt.bfloat16, tag="scrA")
                    sb = scr.tile([128, G, CHUNK // 4], mybir.dt.bfloat16, tag="scrB")
                    bfv = bf.rearrange("p (g f) -> p g f", f=CHUNK)

                    def tmax(o, x, y):
                        nc.vector.tensor_tensor(
                            out=o, in0=x, in1=y, op=mybir.AluOpType.max
                        )

                    # level 1: CHUNK -> CHUNK/2, then ping-pong sa/sb down to
                    # width 1 (last level writes the table slice directly)
                    w = CHUNK // 2
                    tmax(sa[:, :, :w], bfv[:, :, :w], bfv[:, :, w:])
                    cur, other = sa, sb
                    while w > 1:
                        w //= 2
                        if w == 1:
                            dst = tslice.rearrange("p (g f) -> p g f", f=1)
                        else:
                            dst = other[:, :, :w]
                        tmax(dst, cur[:, :, :w], cur[:, :, w : 2 * w])
                        cur, other = other, cur
            nc.sync.dma_start(out=table.ap()[t * 128 : (t + 1) * 128, :], in_=tab)

    nc.compile()
    return nc


def _get_nc():
    if "nc" not in _cached:
        _cached["nc"] = _build_nc()
    return _cached["nc"]


def _bf16(x):
    return np.asarray(x, dtype=np.float64).astype(_BF16)


def _prep_inputs(hand32, obj32):
    """Build per-core lhs/rhs bf16 operands. hand32 [B,NH,3] f32, obj32 [B,NO,3] f32."""
    h64 = hand32.astype(np.float64)
    o64 = obj32.astype(np.float64)

    s64 = 2.0 * h64                                   # exact
    s1 = _bf16(s64)
    s2 = _bf16(s64 - s1.astype(np.float64))
    o1 = _bf16(o64)
    o2 = _bf16(o64 - o1.astype(np.float64))

    ry64 = np.sum(o64 * o64, axis=-1)                 # [B,NO]
    r1 = _bf16(ry64)
    r2 = _bf16(ry64 - r1.astype(np.float64))
    r3 = _bf16(ry64 - r1.astype(np.float64) - r2.astype(np.float64))

    lhs = np.zeros((B, KROWS, NHP), dtype=_BF16)
    rhs = np.zeros((B, KROWS, NOP), dtype=_BF16)

    # object padding: large coords, never the argmin
    PADV = np.float64(1e3)
    opad1 = _bf16(np.array(PADV))
    rypad = 3.0 * PADV * PADV
    rp1 = _bf16(np.array(rypad))
    rhs[:, 0:9:3, NO:] = opad1
    rhs[:, 1:9:3, NO:] = 0
    rhs[:, 2:9:3, NO:] = opad1
    rhs[:, 9, NO:] = rp1

    for c in range(3):
        # product rows per coord: s1*o1 + s1*o2 + s2*o1
        lhs[:, 3 * c + 0, :NH] = s1[..., c]
        lhs[:, 3 * c + 1, :NH] = s1[..., c]
        lhs[:, 3 * c + 2, :NH] = s2[..., c]
        rhs[:, 3 * c + 0, :NO] = o1[..., c]
        rhs[:, 3 * c + 1, :NO] = o2[..., c]
        rhs[:, 3 * c + 2, :NO] = o1[..., c]
    lhs[:, 9:12, :NH] = -1.0
    rhs[:, 9, :NO] = r1
    rhs[:, 10, :NO] = r2
    rhs[:, 11, :NO] = r3
    return lhs, rhs


def _f32(x):
    return np.asarray(x, dtype=np.float32)


def _sumsq32(v):
    """Emulate f32 jnp.sum(v*v, axis=-1) for 3-vectors: (x*x + y*y) + z*z."""
    x = _f32(v[..., 0])
    y = _f32(v[..., 1])
    z = _f32(v[..., 2])
    return (x * x + y * y) + z * z


def _refine_batch(table, h32, o32, rx32, ry32):
    """Host-exact argmin within candidate chunks.

    table [NHP, NCHUNK] f32 (device chunk-max of d'), h32 [NH,3], o32 [NO,3].
    Returns (minho [NH] f32, jstar [NH] int64).
    """
    T = table[:NH].astype(np.float32)                 # [NH, 400]
    best = T.max(axis=1, keepdims=True)
    delta = np.abs(best) * BF16_REL + DELTA           # adaptive band per vertex
    cand = T >= best - delta                          # [NH, 400]
    ridx, cidx = np.nonzero(cand)                     # row-major, cols ascending
    starts = np.searchsorted(ridx, np.arange(NH))
    pos = np.arange(len(ridx)) - starts[ridx]
    C = int(pos.max()) + 1
    candmat = np.full((NH, C), -1, np.int64)
    candmat[ridx, pos] = cidx
    validc = candmat >= 0
    cm = np.maximum(candmat, 0)

    J = cm[:, :, None] * CHUNK + np.arange(CHUNK)[None, None, :]   # [NH,C,128]
    validJ = validc[:, :, None] & (J < NO)
    Jc = np.minimum(J, NO - 1)

    oo = o32[Jc]                                      # [NH,C,128,3] f32
    hh = h32[:, None, None, :]                        # [NH,1,1,3]
    hx, hy, hz = (_f32(hh[..., 0]), _f32(hh[..., 1]), _f32(hh[..., 2]))
    ox, oy, oz = (_f32(oo[..., 0]), _f32(oo[..., 1]), _f32(oo[..., 2]))
    zz = (hx * ox + hy * oy) + hz * oz                # f32 steps
    d = (rx32[:, None, None] + ry32[Jc]) - np.float32(2.0) * zz
    d = np.where(validJ, d, np.float32(np.inf))

    dflat = d.reshape(NH, C * CHUNK)
    jflat = Jc.reshape(NH, C * CHUNK)
    amin = dflat.argmin(axis=1)                       # first occurrence; order is ascending j
    rows = np.arange(NH)
    return dflat[rows, amin], jflat[rows, amin]


def kernel(hand_verts_pt, obj_verts_pt, exterior):
    from concourse.bass_utils import run_bass_kernel_spmd

    hand32 = np.asarray(hand_verts_pt, dtype=np.float32)
    obj32 = np.asarray(obj_verts_pt, dtype=np.float32)
    ext = np.asarray(exterior, dtype=bool)

    lhs, rhs = _prep_inputs(hand32, obj32)
    nc = _get_nc()
    in_maps = [{"lhs": lhs[b], "rhs": rhs[b]} for b in range(B)]
    res = run_bass_kernel_spmd(nc, in_maps, core_ids=list(range(B)))
    tables = [res.results[b]["table"] for b in range(B)]

    rx32 = _sumsq32(hand32)                           # [B,NH] f32 (reference order)
    ry32 = _sumsq32(obj32)                            # [B,NO]

    minho = np.zeros((B, NH), dtype=np.float32)
    results_close = np.zeros((B, NH, 3), dtype=np.float32)
    for b in range(B):
        mv, js = _refine_batch(tables[b], hand32[b], obj32[b], rx32[b], ry32[b])
        minho[b] = mv
        results_close[b] = obj32[b][js]

    # ---- remaining reference math in f32 numpy ----
    penetr_mask = ~ext
    diff = results_close - hand32                     # f32
    anchor_dists = np.sqrt(_sumsq32(diff)).astype(np.float32)   # [B,NH]
    contact_vals = _sumsq32(diff)                     # [B,NH]
    below_dist = minho < np.float32(CONTACT_THRESH**2)
    collision_vals = (
        np.float32(COLLISION_THRESH)
        * np.tanh(anchor_dists / np.float32(COLLISION_THRESH))
    ).astype(np.float32)
    missed_mask = below_dist & ext

    def masked_mean_loss(vals, mask):
        m = mask.astype(np.float32)
        valid = m.sum(dtype=np.float32)
        if valid > 0:
            return np.float32((m * vals).sum(dtype=np.float32) / max(valid, np.float32(1.0)))
        return np.float32(0.0)

    missed_loss = masked_mean_loss(contact_vals, missed_mask)
    penetr_loss = masked_mean_loss(collision_vals, penetr_mask)

    ad = anchor_dists * penetr_mask.astype(np.float32)
    max_penetr_depth = np.float32(ad.max(axis=1).mean(dtype=np.float32))
    mean_penetr_depth = np.float32(ad.mean(axis=1, dtype=np.float32).mean(dtype=np.float32))

    return (
        missed_loss,
        penetr_loss,
        results_close,
        minho,
        max_penetr_depth,
        mean_penetr_depth,
    )
